# revision 27
# baseline (speedup 1.0000x reference)
"""Multi-head attention kernel for Trainium2, SPMD over 8 NeuronCores.

Problem: B=2, S=2048, E=1024, H=16 heads, Dh=64.
  q = per-head q_in @ Wq.T (Wq shared across heads), same for k, v
  attn = softmax(q k^T / 8); ctx = attn @ v; out = concat(ctx) @ Wo.T + bo

Sharding: core c handles batch b=c//4 and heads 4*(c%4)..4*(c%4)+3
(head-parallel attention).  The out projection is sharded by e_out columns
(each core receives 256 rows of Wo), with an AllGather of the per-head
context over the 4 cores of each batch group in between.

Layout strategy (v2):
  - q/k arrive HOST-TRANSPOSED and host-cast to bf16: qT_s/kT_s [256, 2048]
    with head-dim on partitions -> no PE transposes, no staging casts.
    Heads 2g/2g+1 live on partition halves 0-63 / 64-127 of pack g; odd
    heads run their matmuls directly at base partition 64 (PE row-group 64).
  - scores^T = kin @ (A @ qin^T) with A = Wq^T Wk (projection fused).
  - ctx^T unnormalized rides the PE contraction as W2 = vin_ones @ P
    (ones column gives the softmax row-sums for free).
  - Wv is folded into Wo on device (Wo' = Wo @ blockdiag(Wv)): the
    normalized W2 goes straight to the AllGather, no per-head ctx matmul.
  - softmax exp runs on ACT for most tiles; a subset is offloaded to the
    (otherwise idle) DVE via a Schraudolph bit-trick exp in bf16
    (tensor_scalar -> int16 bitcast), balancing the two engines.
  - normalization: DVE reciprocal from PSUM row-sums, GPSIMD partition
    broadcast, DVE fused (W2 * 1/rs) psum->bf16 multiply.

All matmuls run in bf16 with fp32 PSUM accumulation.
"""

import contextlib
import sys

sys.path.insert(0, "/opt/trn_rl_repo")

import numpy as np

import concourse.bass as bass
import concourse.tile as tile
from concourse import bacc, mybir
from concourse.bass_utils import run_bass_kernel_spmd

B, S, E, H, Dh = 2, 2048, 1024, 16, 64
N_CORES = 8
HPC = 4          # heads per core
NK = S // 128    # 16 key chunks
EOUT = E // 4    # e_out rows per core
QH = S // 2      # 1024, q-half width

F32 = mybir.dt.float32
BF16 = mybir.dt.bfloat16
I16 = mybir.dt.int16

# Schraudolph bf16 exp: bitcast_bf16(int16(x * 128/ln2 + (16256 - 128*0.045)))
_EXP_A = 128.0 / float(np.log(2.0))
_EXP_B = 16256.0 - 128.0 * 0.0450

# which m-iterations of each (head, q-half) unit run their exp on DVE
DVE_MS = (2, 4, 7, 9, 12, 15)

_CACHE = {}
_DEBUG = False


def _declare_io(nc):
    io = {}
    if _DEBUG:
        io["dbg_u0"] = nc.dram_tensor("dbg_u0", [128, S], BF16, kind="ExternalOutput").ap()
        io["dbg_p00"] = nc.dram_tensor("dbg_p00", [128, QH], BF16, kind="ExternalOutput").ap()
        io["dbg_w2n00"] = nc.dram_tensor("dbg_w2n00", [Dh, QH], BF16, kind="ExternalOutput").ap()
        io["dbg_w2n10"] = nc.dram_tensor("dbg_w2n10", [Dh, QH], BF16, kind="ExternalOutput").ap()
        io["dbg_rs00"] = nc.dram_tensor("dbg_rs00", [1, QH], F32, kind="ExternalOutput").ap()
        io["dbg_cch"] = nc.dram_tensor("dbg_cch", [128, 8 * S], BF16, kind="ExternalOutput").ap()
        io["dbg_woF"] = nc.dram_tensor("dbg_woF", [128, 8 * EOUT], BF16, kind="ExternalOutput").ap()
        io["dbg_w2raw"] = nc.dram_tensor("dbg_w2raw", [Dh + 1, QH], F32, kind="ExternalOutput").ap()
        io["dbg_vones"] = nc.dram_tensor("dbg_vones", [128, NK * HPC * (Dh + 1)], BF16, kind="ExternalOutput").ap()
    io["qT_s"] = nc.dram_tensor("qT_s", [2 * 128, S], BF16, kind="ExternalInput").ap()
    io["kT_s"] = nc.dram_tensor("kT_s", [2 * 128, S], BF16, kind="ExternalInput").ap()
    io["vin"] = nc.dram_tensor("vin", [S, HPC * Dh], BF16, kind="ExternalInput").ap()
    io["wq"] = nc.dram_tensor("wq", [Dh, Dh], F32, kind="ExternalInput").ap()
    io["wk"] = nc.dram_tensor("wk", [Dh, Dh], F32, kind="ExternalInput").ap()
    io["wv"] = nc.dram_tensor("wv", [Dh, Dh], F32, kind="ExternalInput").ap()
    io["woT_s"] = nc.dram_tensor("woT_s", [E, EOUT], BF16, kind="ExternalInput").ap()
    io["bo_s"] = nc.dram_tensor("bo_s", [2, 128], F32, kind="ExternalInput").ap()
    io["outT"] = nc.dram_tensor("outT", [EOUT, S], F32, kind="ExternalOutput").ap()
    return io


def _body(nc, tc, es, io, it, collective=True):
    """One full MHA iteration. `it` only namespaces pool names."""

    def pool(name, bufs, space="SBUF"):
        return es.enter_context(
            tc.tile_pool(name=f"{name}_{it}", bufs=bufs, space=space)
        )

    qT_s, kT_s, vin = io["qT_s"], io["kT_s"], io["vin"]
    wq, wk, wv, woT_s, bo_s, outT = (
        io["wq"], io["wk"], io["wv"], io["woT_s"], io["bo_s"], io["outT"],
    )

    persist = pool("persist", 1)      # long-lived bf16 tensors
    ppool = pool("ppool", 8)          # exp outputs
    npool = pool("npool", 2)          # normalize chain tiles
    opool = pool("opool", 2)          # out-projection sbuf tiles
    psum_big = pool("psum_big", 3, space="PSUM")   # [128,1024] x3 = 6 banks
    psum_acc = pool("psum_acc", 1, space="PSUM")   # [65,1024]  x1 = 2 banks
    dram = pool("dram", 1, space="DRAM")

    # ---------------- persistent tiles ----------------
    qT = [persist.tile([128, S], BF16, tag=f"qT{g}", name=f"qT{g}") for g in range(2)]
    kT = [persist.tile([128, S], BF16, tag=f"kT{g}", name=f"kT{g}") for g in range(2)]
    u = [persist.tile([128, S], BF16, tag=f"u{g}", name=f"u{g}") for g in range(2)]
    vin_ones = persist.tile([128, NK, HPC, Dh + 1], BF16, tag="vin_ones")
    a2 = persist.tile([128, Dh], BF16, tag="a2")      # A on both partition halves
    wv2 = persist.tile([128, Dh], BF16, tag="wv2")    # Wv on both partition halves
    woTp = persist.tile([128, 8, EOUT], BF16, tag="woTp")   # WoT slice, raw
    woF = persist.tile([128, 8, EOUT], BF16, tag="woF")     # blockdiag(Wv^T) @ WoT
    bo_sb = persist.tile([128, 2], F32, tag="bo_sb")
    o_acc = [opool.tile([128, S], F32, tag=f"oacc{h}", bufs=1, name=f"oacc{h}")
             for h in range(2)]

    # ---------------- prologue DMAs + tiny weight prep ----------------
    # prologue loads split across the SP and ACT DGE queues (HWDGE
    # generation is ~630ns serialized per queue); tiny weight DMAs lead so
    # the A-matrix chain overlaps the big k/q loads
    wq_sb = persist.tile([Dh, Dh], F32, tag="wq_sb")
    nc.sync.dma_start(out=wq_sb[:], in_=wq[:, :])
    nc.scalar.dma_start(out=kT[0][:, 0:QH], in_=kT_s[0:128, 0:QH])
    wk_sb = persist.tile([Dh, Dh], F32, tag="wk_sb")
    nc.sync.dma_start(out=wk_sb[:], in_=wk[:, :])
    nc.scalar.dma_start(out=qT[0][:, 0:QH], in_=qT_s[0:128, 0:QH])

    wq_bf = persist.tile([Dh, Dh], BF16, tag="wq_bf")
    nc.vector.tensor_copy(wq_bf[:], wq_sb[:])
    wk_bf = persist.tile([Dh, Dh], BF16, tag="wk_bf")
    nc.vector.tensor_copy(wk_bf[:], wk_sb[:])

    # A = Wq^T @ Wk  [64,64]; replicate to partitions 64-127 via small DMA
    # on the ACT DGE queue (so it never blocks the activation-load FIFO)
    a_ps = psum_big.tile([128, 1024], F32, tag="big", name=f"aps_{it}")
    nc.tensor.matmul(a_ps[0:Dh, 0:Dh], wq_bf[:], wk_bf[:], start=True, stop=True)
    nc.vector.tensor_copy(a2[0:Dh, :], a_ps[0:Dh, 0:Dh])
    nc.scalar.dma_start(out=a2[Dh : 2 * Dh, :], in_=a2[0:Dh, :])

    def emit_vin(j):
        nc.sync.dma_start(
            out=vin_ones[:, :, j, 0:Dh],
            in_=vin[:, Dh * j : Dh * (j + 1)].rearrange("(c p) d -> p c d", p=128),
        )

    emit_vin(0)
    nc.vector.memset(vin_ones[:, :, :, Dh : Dh + 1], 1.0)

    wv_sb = persist.tile([Dh, Dh], F32, tag="wv_sb")
    nc.scalar.dma_start(out=wv_sb[:], in_=wv[:, :])
    nc.vector.tensor_copy(wv2[0:Dh, :], wv_sb[:])
    nc.scalar.dma_start(out=wv2[Dh : 2 * Dh, :], in_=wv2[0:Dh, :])
    for h in range(2):
        nc.scalar.dma_start(
            out=bo_sb[:, h : h + 1],
            in_=bo_s[h, :].rearrange("(p one) -> p one", one=1),
        )

    # remaining activation loads, emitted inside the m-loop hooks below
    def emit_kq_rest0():
        nc.sync.dma_start(out=kT[0][:, QH:S], in_=kT_s[0:128, QH:S])
        nc.sync.dma_start(out=qT[0][:, QH:S], in_=qT_s[0:128, QH:S])

    def emit_kq1(half):
        sl = slice(QH * half, QH * (half + 1))
        nc.sync.dma_start(out=kT[1][:, sl], in_=kT_s[128:256, sl])
        nc.sync.dma_start(out=qT[1][:, sl], in_=qT_s[128:256, sl])

    def emit_wot_load():
        nc.sync.dma_start(
            out=woTp[:], in_=woT_s[:, :].rearrange("(c p) e -> p c e", p=128)
        )

    def emit_u(g, qh, half=None):
        """u[g][:, qh half] = A^T @ qT for heads 2g (parts 0-63) and 2g+1
        (parts 64-127, diagonal PE tile). `half` restricts to one partition
        half (used in the prologue so head 0 starts before the a2 replica
        DMA lands)."""
        u_ps = psum_big.tile([128, 1024], F32, tag="big", name=f"ups_{it}_{g}_{qh}_{half}")
        halves = (0, 1) if half is None else (half,)
        for t in range(2):
            csl = slice(QH * qh + 512 * t, QH * qh + 512 * (t + 1))
            osl = slice(512 * t, 512 * (t + 1))
            for hf in halves:
                hsl = slice(Dh * hf, Dh * (hf + 1))
                nc.tensor.matmul(
                    u_ps[hsl, osl], a2[hsl, :], qT[g][hsl, csl],
                    start=True, stop=True,
                )
        if half is None:
            nc.vector.tensor_copy(u[g][:, QH * qh : QH * (qh + 1)], u_ps[:])
        else:
            hsl = slice(Dh * half, Dh * (half + 1))
            nc.vector.tensor_copy(u[g][hsl, QH * qh : QH * (qh + 1)], u_ps[hsl, :])

    def emit_woF(half):
        """woF chunks 4*half..4*half+3 = blockdiag(Wv^T) @ woTp chunks."""
        f_ps = psum_big.tile([128, 1024], F32, tag="big", name=f"wfps_{it}_{half}")
        for i in range(4):
            c8 = 4 * half + i
            osl = slice(EOUT * i, EOUT * (i + 1))
            nc.tensor.matmul(
                f_ps[0:Dh, osl], wv2[0:Dh, :], woTp[0:Dh, c8, :],
                start=True, stop=True,
            )
            nc.tensor.matmul(
                f_ps[Dh:128, osl], wv2[Dh:128, :], woTp[Dh:128, c8, :],
                start=True, stop=True,
            )
        nc.vector.tensor_copy(
            woF[:, 4 * half : 4 * (half + 1), :].rearrange("p c e -> p (c e)"),
            f_ps[:],
        )

    # ---------------- AllGather staging ----------------
    # pair (pr, qh) staging; the final (1, 1) round is split into two q-512
    # chunks so its serial normalize -> AG -> cch -> oproj chain pipelines.
    in_cc = [
        [dram.tile([2 * Dh, QH], BF16, name=f"incc_{it}_{pr}_{qh}", tag=f"incc{pr}{qh}")
         for qh in range(2)]
        for pr in range(2)
    ]
    ag_outs = [
        [dram.tile([512, QH], BF16, addr_space="Local",
                   name=f"agout_{it}_{pr}_{qh}", tag=f"agout{pr}{qh}")
         for qh in range(2)]
        for pr in range(2)
    ]
    in_cc11 = [
        dram.tile([2 * Dh, 512], BF16, name=f"incc11_{it}_{qc}", tag=f"incc11{qc}")
        for qc in range(2)
    ]
    ag11 = [
        dram.tile([512, 512], BF16, addr_space="Local",
                  name=f"ag11_{it}_{qc}", tag=f"ag11{qc}")
        for qc in range(2)
    ]
    # one tile per 128-row concat chunk: keeps dependency tracking
    # per-chunk so late AllGathers never serialize earlier oproj reads
    cch = [persist.tile([128, S], BF16, tag=f"cch{c8}", name=f"cch{c8}_{it}")
           for c8 in range(8)]

    def emit_ag(pr, qh):
        if collective:
            nc.gpsimd.collective_compute(
                "AllGather",
                mybir.AluOpType.bypass,
                replica_groups=[[0, 1, 2, 3], [4, 5, 6, 7]],
                ins=[in_cc[pr][qh][:, :].opt()],
                outs=[ag_outs[pr][qh].opt()],
            )
        else:
            # sim stand-in: a light dep edge; real AG runs on TOPSP silicon
            nc.sync.dma_start(out=ag_outs[pr][qh][0:128, :], in_=in_cc[pr][qh][:, :])
        for r in range(4):
            eng = (nc.sync, nc.scalar)[r % 2]
            eng.dma_start(
                out=cch[2 * r + pr][:, QH * qh : QH * (qh + 1)],
                in_=ag_outs[pr][qh][128 * r : 128 * (r + 1), :],
            )

    def emit_ag11(qc):
        """Final AllGather, q-512 chunk qc of q-half 1; cch loads spread over
        the SP and DVE DGE queues (both idle in the tail)."""
        if collective:
            nc.gpsimd.collective_compute(
                "AllGather",
                mybir.AluOpType.bypass,
                replica_groups=[[0, 1, 2, 3], [4, 5, 6, 7]],
                ins=[in_cc11[qc][:, :].opt()],
                outs=[ag11[qc].opt()],
            )
        else:
            nc.sync.dma_start(out=ag11[qc][0:128, :], in_=in_cc11[qc][:, :])
        for r in range(4):
            eng = (nc.sync, nc.scalar, nc.gpsimd, nc.sync)[r]
            eng.dma_start(
                out=cch[2 * r + 1][:, QH + 512 * qc : QH + 512 * (qc + 1)],
                in_=ag11[qc][128 * r : 128 * (r + 1), :],
            )

    # ---------------- out projection ----------------
    def emit_oproj(pr, qh):
        """Accumulate parity-`pr` chunks of q-half `qh` into o_acc (pr 0) or
        finish with bias into outT (pr 1)."""
        for h in range(2):
            o_ps = psum_big.tile([128, 1024], F32, tag="big", name=f"ops_{it}_{pr}_{qh}_{h}")
            for t in range(2):
                osl = slice(512 * t, 512 * (t + 1))
                for i, r in enumerate(range(4)):
                    c8 = 2 * r + pr
                    nc.tensor.matmul(
                        o_ps[:, osl],
                        woF[:, c8, 128 * h : 128 * (h + 1)],
                        cch[c8][:, QH * qh + 512 * t : QH * qh + 512 * (t + 1)],
                        start=(i == 0), stop=(i == 3),
                    )
            qsl = slice(QH * qh, QH * (qh + 1))
            if pr == 0:
                nc.vector.tensor_copy(o_acc[h][:, qsl], o_ps[:])
            else:
                o_sb = opool.tile([128, 1024], F32, tag="osb", name=f"osb_{it}_{qh}_{h}")
                nc.vector.scalar_tensor_tensor(
                    o_sb[:], o_ps[:], bo_sb[:, h : h + 1], o_acc[h][:, qsl],
                    mybir.AluOpType.add, mybir.AluOpType.add,
                )
                nc.sync.dma_start(
                    out=outT[128 * h : 128 * (h + 1), qsl], in_=o_sb[:]
                )

    # ---------------- attention stream ----------------
    # All 8 (head, q-half) units flatten into one global stream of 128
    # iterations: sc(i) + exp(i) at iteration i, with the W2 consumer lagging
    # GLOBALLY by W2_LAG iterations so the in-order PE queue never waits on a
    # just-finished exp (the old lag-1 scheme serialized
    # exp -> W2 -> sc -> exp across engines; lag 3 also rides out DVE FIFO latency).
    W2_LAG = 3

    def emit_normalize(j, qh, w2_ps):
        """Normalize chain (executes overlapped with later iterations).
        The row-sum row sits on PSUM partition 64; only ACT can move it to
        partition 0 (DVE lanes are partition-locked, GPSIMD broadcast always
        reads partition 0, DMA cannot read PSUM)."""
        odd = j % 2
        if _DEBUG and j == 0 and qh == 0:
            wraw = persist.tile([Dh + 1, QH], F32, tag="dbg_w2raw_t", name=f"dbgwr_{it}")
            nc.vector.tensor_copy(wraw[:], w2_ps[:, :])
            nc.sync.dma_start(out=io["dbg_w2raw"][:, :], in_=wraw[:])
        rs0 = npool.tile([1, QH], F32, tag="rs0", name=f"rs0_{it}_{j}_{qh}")
        nc.scalar.copy(rs0[:], w2_ps[Dh : Dh + 1, :])
        rsr = npool.tile([1, QH], F32, tag="rsr", name=f"rsr_{it}_{j}_{qh}")
        nc.vector.reciprocal_approx_fast(out=rsr[:], in_=rs0[:])
        rs_b = npool.tile([Dh, QH], F32, tag="rs_b", name=f"rsb_{it}_{j}_{qh}")
        nc.gpsimd.partition_broadcast(rs_b[:], rsr[:])
        w2n = npool.tile([Dh, QH], BF16, tag="w2n", name=f"w2n_{it}_{j}_{qh}")
        nc.vector.tensor_tensor(
            w2n[:], w2_ps[0:Dh, :], rs_b[:], mybir.AluOpType.mult
        )
        if j == 2 and qh == 1:
            # head 2's q-half 1 feeds the SPLIT final-AG staging tiles
            for qc in range(2):
                nc.sync.dma_start(
                    out=in_cc11[qc][0:Dh, :],
                    in_=w2n[:, 512 * qc : 512 * (qc + 1)],
                )
        else:
            nc.sync.dma_start(
                out=in_cc[j // 2][qh][Dh * odd : Dh * (odd + 1), :], in_=w2n[:]
            )
        if _DEBUG and qh == 0 and j in (0, 1):
            nc.sync.dma_start(out=io[f"dbg_w2n{j}0"][:, :], in_=w2n[:])
            if j == 0:
                nc.sync.dma_start(out=io["dbg_rs00"][:, :], in_=rsr[:])
        if odd == 1 and not (j == 3 and qh == 1):
            emit_ag(j // 2, qh)

    def emit_tail(w2_ps):
        """Final unit (head 3, q-half 1): normalize, AllGather, and the last
        out-projection round all split into q-512 chunks so the serial chain
        pipelines; the other three oproj rounds fill the PE meanwhile."""
        o_ps_h = [
            psum_big.tile([128, 1024], F32, tag="big", name=f"opsT_{it}_{h}")
            for h in range(2)
        ]
        for qc in range(2):
            csl = slice(512 * qc, 512 * (qc + 1))
            rs0 = npool.tile([1, 512], F32, tag=f"rs0T{qc}", name=f"rs0T_{it}_{qc}")
            nc.scalar.copy(rs0[:], w2_ps[Dh : Dh + 1, csl])
            rsr = npool.tile([1, 512], F32, tag=f"rsrT{qc}", name=f"rsrT_{it}_{qc}")
            nc.vector.reciprocal_approx_fast(out=rsr[:], in_=rs0[:])
            rs_b = npool.tile([Dh, 512], F32, tag=f"rsbT{qc}", name=f"rsbT_{it}_{qc}")
            nc.gpsimd.partition_broadcast(rs_b[:], rsr[:])
            w2n = npool.tile([Dh, 512], BF16, tag=f"w2nT{qc}", name=f"w2nT_{it}_{qc}")
            nc.vector.tensor_tensor(
                w2n[:], w2_ps[0:Dh, csl], rs_b[:], mybir.AluOpType.mult
            )
            eng = nc.sync if qc == 0 else nc.scalar
            eng.dma_start(out=in_cc11[qc][Dh : 2 * Dh, :], in_=w2n[:])
            emit_ag11(qc)
            if qc == 0:
                emit_oproj(0, 0)
                emit_oproj(0, 1)
                emit_oproj(1, 0)
            for h in range(2):
                for i, r in enumerate(range(4)):
                    nc.tensor.matmul(
                        o_ps_h[h][:, csl],
                        woF[:, 2 * r + 1, 128 * h : 128 * (h + 1)],
                        cch[2 * r + 1][:, QH + 512 * qc : QH + 512 * (qc + 1)],
                        start=(i == 0), stop=(i == 3),
                    )
                o_sb = opool.tile(
                    [128, 512], F32, tag=f"osbT{qc}", name=f"osbT_{it}_{qc}_{h}"
                )
                nc.vector.scalar_tensor_tensor(
                    o_sb[:], o_ps_h[h][:, csl], bo_sb[:, h : h + 1],
                    o_acc[h][:, QH + 512 * qc : QH + 512 * (qc + 1)],
                    mybir.AluOpType.add, mybir.AluOpType.add,
                )
                eng = nc.sync if h == 0 else nc.scalar
                eng.dma_start(
                    out=outT[128 * h : 128 * (h + 1), QH + 512 * qc : QH + 512 * (qc + 1)],
                    in_=o_sb[:],
                )

    # ---------------- schedule ----------------
    emit_u(0, 0, half=0)

    hooks = {
        (0, 0): {1: lambda: emit_u(0, 0, half=1),
                 2: emit_kq_rest0, 6: lambda: emit_vin(1), 10: lambda: emit_u(0, 1)},
        (0, 1): {2: lambda: emit_kq1(0), 6: emit_wot_load, 10: lambda: emit_vin(2)},
        (1, 0): {2: lambda: emit_kq1(1), 5: lambda: emit_u(1, 0),
                 9: lambda: emit_woF(0), 12: lambda: emit_woF(1)},
        (1, 1): {2: lambda: emit_vin(3), 6: lambda: emit_u(1, 1)},
        (2, 0): {},
        (2, 1): {},
        (3, 0): {},
        (3, 1): {},
    }

    units = [(j, qh) for j in range(HPC) for qh in range(2)]
    unit_state = {}        # k -> w2_ps tile
    pend = []              # (emit_w2_closure, post_closure_or_None)

    def drain_one():
        w2c, post = pend.pop(0)
        w2c()
        if post is not None:
            post()

    for k, (j, qh) in enumerate(units):
        g, odd = j // 2, j % 2
        psl = slice(Dh * odd, Dh * (odd + 1))
        w2_ps = psum_acc.tile([Dh + 1, QH], F32, tag="acc", name=f"w2ps_{it}_{j}_{qh}")
        for m in range(NK):
            hk = hooks[(j, qh)].get(m)
            if hk is not None:
                hk()
            sc_ps = psum_big.tile([128, 1024], F32, tag="big", name=f"scps_{it}_{j}_{qh}_{m}")
            for t in range(2):
                nc.tensor.matmul(
                    sc_ps[:, 512 * t : 512 * (t + 1)],
                    kT[g][psl, 128 * m : 128 * (m + 1)],
                    u[g][psl, QH * qh + 512 * t : QH * qh + 512 * (t + 1)],
                    start=True, stop=True,
                )
            p_bf = ppool.tile([128, 1024], BF16, tag="p", name=f"p_{it}_{j}_{qh}_{m}")
            if m in DVE_MS:
                nc.vector.tensor_scalar(
                    p_bf[:].bitcast(I16), sc_ps[:],
                    _EXP_A * 0.125, _EXP_B,
                    mybir.AluOpType.mult, mybir.AluOpType.add,
                )
            else:
                nc.scalar.activation(
                    p_bf[:], sc_ps[:], mybir.ActivationFunctionType.Exp, scale=0.125
                )
            if _DEBUG and j == 0 and qh == 0 and m == 0:
                nc.sync.dma_start(out=io["dbg_p00"][:, :], in_=p_bf[:])

            def w2c(w2_ps=w2_ps, m=m, j=j, p_bf=p_bf):
                for t2 in range(2):
                    nc.tensor.matmul(
                        w2_ps[:, 512 * t2 : 512 * (t2 + 1)],
                        vin_ones[:, m, j, :],
                        p_bf[:, 512 * t2 : 512 * (t2 + 1)],
                        start=(m == 0), stop=(m == NK - 1),
                    )

            post = None
            if m == NK - 1:
                if (j, qh) == (HPC - 1, 1):
                    def post(w2_ps=w2_ps):
                        emit_tail(w2_ps)
                else:
                    def post(j=j, qh=qh, w2_ps=w2_ps):
                        emit_normalize(j, qh, w2_ps)
            pend.append((w2c, post))
            if len(pend) > W2_LAG:
                drain_one()
        # single-buffered W2 psum: drain this unit's W2 tail + normalize NOW
        # so the chain completes before the next unit's first W2 needs the
        # buffer (costs a short PE wait on the last exp, once per unit).
        while pend:
            drain_one()
    if _DEBUG:
        nc.sync.dma_start(out=io["dbg_u0"][:, :], in_=u[0][:, :])
        for c8 in range(8):
            nc.sync.dma_start(
                out=io["dbg_cch"][:, S * c8 : S * (c8 + 1)], in_=cch[c8][:, :]
            )
        nc.sync.dma_start(
            out=io["dbg_woF"][:, :],
            in_=woF[:, :, :].rearrange("p c e -> p (c e)"),
        )
        nc.sync.dma_start(
            out=io["dbg_vones"][:, :],
            in_=vin_ones[:, :, :, :].rearrange("p c j d -> p (c j d)"),
        )


def _build(repeats=1, collective=True):
    key = (repeats, collective)
    if key in _CACHE:
        return _CACHE[key]
    ndev = N_CORES if collective else 1
    nc = bacc.Bacc("TRN2", target_bir_lowering=False, debug=False, num_devices=ndev)
    io = _declare_io(nc)
    with tile.TileContext(nc) as tc:
        for it in range(repeats):
            with contextlib.ExitStack() as es:
                _body(nc, tc, es, io, it, collective=collective)
    nc.compile()
    _CACHE[key] = nc
    return nc


def kernel(k_in, q_in, v_in, Wq, Wk, Wv, Wo, bo, _repeats=1, _results_hook=None):
    import ml_dtypes

    bf16 = ml_dtypes.bfloat16
    k_in = np.asarray(k_in, dtype=np.float32)
    q_in = np.asarray(q_in, dtype=np.float32)
    v_in = np.asarray(v_in, dtype=np.float32)
    Wq = np.ascontiguousarray(np.asarray(Wq, dtype=np.float32))
    Wk = np.ascontiguousarray(np.asarray(Wk, dtype=np.float32))
    Wv = np.ascontiguousarray(np.asarray(Wv, dtype=np.float32))
    Wo = np.asarray(Wo, dtype=np.float32)
    bo = np.asarray(bo, dtype=np.float32)

    nc = _build(_repeats)

    in_maps = []
    for c in range(N_CORES):
        b, q4 = c // 4, c % 4
        sl = slice(256 * q4, 256 * (q4 + 1))
        in_maps.append(
            {
                "qT_s": q_in[b, :, sl].T.astype(bf16),
                "kT_s": k_in[b, :, sl].T.astype(bf16),
                "vin": v_in[b, :, sl].astype(bf16),
                "wq": Wq,
                "wk": Wk,
                "wv": Wv,
                "woT_s": Wo[sl, :].T.astype(bf16),
                "bo_s": np.ascontiguousarray(bo[sl].reshape(2, 128)),
            }
        )

    res = run_bass_kernel_spmd(nc, in_maps, core_ids=list(range(N_CORES)))
    if _results_hook is not None:
        _results_hook(res)

    out = np.empty((B, S, E), dtype=np.float32)
    for c in range(N_CORES):
        b, q4 = c // 4, c % 4
        out[b, :, 256 * q4 : 256 * (q4 + 1)] = res.results[c]["outT"].T
    return out


# revision 28
# speedup vs baseline: 1.0069x; 1.0069x over previous
"""Multi-head attention kernel for Trainium2, SPMD over 8 NeuronCores.

Problem: B=2, S=2048, E=1024, H=16 heads, Dh=64.
  q = per-head q_in @ Wq.T (Wq shared across heads), same for k, v
  attn = softmax(q k^T / 8); ctx = attn @ v; out = concat(ctx) @ Wo.T + bo

Sharding: core c handles batch b=c//4 and heads 4*(c%4)..4*(c%4)+3
(head-parallel attention).  The out projection is sharded by e_out columns
(each core receives 256 rows of Wo), with an AllGather of the per-head
context over the 4 cores of each batch group in between.

Layout strategy (v2):
  - q/k arrive HOST-TRANSPOSED and host-cast to bf16: qT_s/kT_s [256, 2048]
    with head-dim on partitions -> no PE transposes, no staging casts.
    Heads 2g/2g+1 live on partition halves 0-63 / 64-127 of pack g; odd
    heads run their matmuls directly at base partition 64 (PE row-group 64).
  - scores^T = kin @ (A @ qin^T) with A = Wq^T Wk (projection fused).
  - ctx^T unnormalized rides the PE contraction as W2 = vin_ones @ P
    (ones column gives the softmax row-sums for free).
  - Wv is folded into Wo on device (Wo' = Wo @ blockdiag(Wv)): the
    normalized W2 goes straight to the AllGather, no per-head ctx matmul.
  - softmax exp runs on ACT for most tiles; a subset is offloaded to the
    (otherwise idle) DVE via a Schraudolph bit-trick exp in bf16
    (tensor_scalar -> int16 bitcast), balancing the two engines.
  - normalization: DVE reciprocal from PSUM row-sums, GPSIMD partition
    broadcast, DVE fused (W2 * 1/rs) psum->bf16 multiply.

All matmuls run in bf16 with fp32 PSUM accumulation.
"""

import contextlib
import sys

sys.path.insert(0, "/opt/trn_rl_repo")

import numpy as np

import concourse.bass as bass
import concourse.tile as tile
from concourse import bacc, mybir
from concourse.bass_utils import run_bass_kernel_spmd

B, S, E, H, Dh = 2, 2048, 1024, 16, 64
N_CORES = 8
HPC = 4          # heads per core
NK = S // 128    # 16 key chunks
EOUT = E // 4    # e_out rows per core
QH = S // 2      # 1024, q-half width

F32 = mybir.dt.float32
BF16 = mybir.dt.bfloat16
I16 = mybir.dt.int16

# Schraudolph bf16 exp: bitcast_bf16(int16(x * 128/ln2 + (16256 - 128*0.045)))
_EXP_A = 128.0 / float(np.log(2.0))
_EXP_B = 16256.0 - 128.0 * 0.0450

# which m-iterations of each (head, q-half) unit run their exp on DVE
DVE_MS = (2, 4, 7, 9, 12, 14)

_CACHE = {}
_DEBUG = False


def _declare_io(nc):
    io = {}
    if _DEBUG:
        io["dbg_u0"] = nc.dram_tensor("dbg_u0", [128, S], BF16, kind="ExternalOutput").ap()
        io["dbg_p00"] = nc.dram_tensor("dbg_p00", [128, QH], BF16, kind="ExternalOutput").ap()
        io["dbg_w2n00"] = nc.dram_tensor("dbg_w2n00", [Dh, QH], BF16, kind="ExternalOutput").ap()
        io["dbg_w2n10"] = nc.dram_tensor("dbg_w2n10", [Dh, QH], BF16, kind="ExternalOutput").ap()
        io["dbg_rs00"] = nc.dram_tensor("dbg_rs00", [1, QH], F32, kind="ExternalOutput").ap()
        io["dbg_cch"] = nc.dram_tensor("dbg_cch", [128, 8 * S], BF16, kind="ExternalOutput").ap()
        io["dbg_woF"] = nc.dram_tensor("dbg_woF", [128, 8 * EOUT], BF16, kind="ExternalOutput").ap()
        io["dbg_w2raw"] = nc.dram_tensor("dbg_w2raw", [Dh + 1, QH], F32, kind="ExternalOutput").ap()
        io["dbg_vones"] = nc.dram_tensor("dbg_vones", [128, NK * HPC * (Dh + 1)], BF16, kind="ExternalOutput").ap()
    io["qT_s"] = nc.dram_tensor("qT_s", [2 * 128, S], BF16, kind="ExternalInput").ap()
    io["kT_s"] = nc.dram_tensor("kT_s", [2 * 128, S], BF16, kind="ExternalInput").ap()
    io["vin"] = nc.dram_tensor("vin", [S, HPC * Dh], BF16, kind="ExternalInput").ap()
    io["wq"] = nc.dram_tensor("wq", [Dh, Dh], F32, kind="ExternalInput").ap()
    io["wk"] = nc.dram_tensor("wk", [Dh, Dh], F32, kind="ExternalInput").ap()
    io["wv"] = nc.dram_tensor("wv", [Dh, Dh], F32, kind="ExternalInput").ap()
    io["woT_s"] = nc.dram_tensor("woT_s", [E, EOUT], BF16, kind="ExternalInput").ap()
    io["bo_s"] = nc.dram_tensor("bo_s", [2, 128], F32, kind="ExternalInput").ap()
    io["outT"] = nc.dram_tensor("outT", [EOUT, S], F32, kind="ExternalOutput").ap()
    return io


def _body(nc, tc, es, io, it, collective=True):
    """One full MHA iteration. `it` only namespaces pool names."""

    def pool(name, bufs, space="SBUF"):
        return es.enter_context(
            tc.tile_pool(name=f"{name}_{it}", bufs=bufs, space=space)
        )

    qT_s, kT_s, vin = io["qT_s"], io["kT_s"], io["vin"]
    wq, wk, wv, woT_s, bo_s, outT = (
        io["wq"], io["wk"], io["wv"], io["woT_s"], io["bo_s"], io["outT"],
    )

    persist = pool("persist", 1)      # long-lived bf16 tensors
    ppool = pool("ppool", 8)          # exp outputs
    npool = pool("npool", 2)          # normalize chain tiles
    opool = pool("opool", 2)          # out-projection sbuf tiles
    psum_big = pool("psum_big", 3, space="PSUM")   # [128,1024] x3 = 6 banks
    psum_acc = pool("psum_acc", 1, space="PSUM")   # [65,1024]  x1 = 2 banks
    dram = pool("dram", 1, space="DRAM")

    # ---------------- persistent tiles ----------------
    qT = [persist.tile([128, S], BF16, tag=f"qT{g}", name=f"qT{g}") for g in range(2)]
    kT = [persist.tile([128, S], BF16, tag=f"kT{g}", name=f"kT{g}") for g in range(2)]
    u = [persist.tile([128, S], BF16, tag=f"u{g}", name=f"u{g}") for g in range(2)]
    vin_ones = persist.tile([128, NK, HPC, Dh + 1], BF16, tag="vin_ones")
    a2 = persist.tile([128, Dh], BF16, tag="a2")      # A on both partition halves
    wv2 = persist.tile([128, Dh], BF16, tag="wv2")    # Wv on both partition halves
    woTp = persist.tile([128, 8, EOUT], BF16, tag="woTp")   # WoT slice, raw
    woF = persist.tile([128, 8, EOUT], BF16, tag="woF")     # blockdiag(Wv^T) @ WoT
    bo_sb = persist.tile([128, 2], F32, tag="bo_sb")
    o_acc = [opool.tile([128, S], F32, tag=f"oacc{h}", bufs=1, name=f"oacc{h}")
             for h in range(2)]

    # ---------------- prologue DMAs + tiny weight prep ----------------
    # prologue loads split across the SP and ACT DGE queues (HWDGE
    # generation is ~630ns serialized per queue); tiny weight DMAs lead so
    # the A-matrix chain overlaps the big k/q loads
    wq_sb = persist.tile([Dh, Dh], F32, tag="wq_sb")
    nc.sync.dma_start(out=wq_sb[:], in_=wq[:, :])
    nc.scalar.dma_start(out=kT[0][:, 0:QH], in_=kT_s[0:128, 0:QH])
    wk_sb = persist.tile([Dh, Dh], F32, tag="wk_sb")
    nc.sync.dma_start(out=wk_sb[:], in_=wk[:, :])
    nc.scalar.dma_start(out=qT[0][:, 0:QH], in_=qT_s[0:128, 0:QH])

    wq_bf = persist.tile([Dh, Dh], BF16, tag="wq_bf")
    nc.vector.tensor_copy(wq_bf[:], wq_sb[:])
    wk_bf = persist.tile([Dh, Dh], BF16, tag="wk_bf")
    nc.vector.tensor_copy(wk_bf[:], wk_sb[:])

    # A = Wq^T @ Wk  [64,64]; replicate to partitions 64-127 via small DMA
    # on the ACT DGE queue (so it never blocks the activation-load FIFO)
    a_ps = psum_big.tile([128, 1024], F32, tag="big", name=f"aps_{it}")
    nc.tensor.matmul(a_ps[0:Dh, 0:Dh], wq_bf[:], wk_bf[:], start=True, stop=True)
    nc.vector.tensor_copy(a2[0:Dh, :], a_ps[0:Dh, 0:Dh])
    nc.scalar.dma_start(out=a2[Dh : 2 * Dh, :], in_=a2[0:Dh, :])

    def emit_vin(j):
        nc.sync.dma_start(
            out=vin_ones[:, :, j, 0:Dh],
            in_=vin[:, Dh * j : Dh * (j + 1)].rearrange("(c p) d -> p c d", p=128),
        )

    emit_vin(0)
    nc.vector.memset(vin_ones[:, :, :, Dh : Dh + 1], 1.0)

    wv_sb = persist.tile([Dh, Dh], F32, tag="wv_sb")
    nc.scalar.dma_start(out=wv_sb[:], in_=wv[:, :])
    nc.vector.tensor_copy(wv2[0:Dh, :], wv_sb[:])
    nc.scalar.dma_start(out=wv2[Dh : 2 * Dh, :], in_=wv2[0:Dh, :])
    for h in range(2):
        nc.scalar.dma_start(
            out=bo_sb[:, h : h + 1],
            in_=bo_s[h, :].rearrange("(p one) -> p one", one=1),
        )

    # remaining activation loads, emitted inside the m-loop hooks below
    def emit_kq_rest0():
        nc.sync.dma_start(out=kT[0][:, QH:S], in_=kT_s[0:128, QH:S])
        nc.sync.dma_start(out=qT[0][:, QH:S], in_=qT_s[0:128, QH:S])

    def emit_kq1(half):
        sl = slice(QH * half, QH * (half + 1))
        nc.sync.dma_start(out=kT[1][:, sl], in_=kT_s[128:256, sl])
        nc.sync.dma_start(out=qT[1][:, sl], in_=qT_s[128:256, sl])

    def emit_wot_load():
        nc.sync.dma_start(
            out=woTp[:], in_=woT_s[:, :].rearrange("(c p) e -> p c e", p=128)
        )

    def emit_u(g, qh, half=None):
        """u[g][:, qh half] = A^T @ qT for heads 2g (parts 0-63) and 2g+1
        (parts 64-127, diagonal PE tile). `half` restricts to one partition
        half (used in the prologue so head 0 starts before the a2 replica
        DMA lands)."""
        u_ps = psum_big.tile([128, 1024], F32, tag="big", name=f"ups_{it}_{g}_{qh}_{half}")
        halves = (0, 1) if half is None else (half,)
        for t in range(2):
            csl = slice(QH * qh + 512 * t, QH * qh + 512 * (t + 1))
            osl = slice(512 * t, 512 * (t + 1))
            for hf in halves:
                hsl = slice(Dh * hf, Dh * (hf + 1))
                nc.tensor.matmul(
                    u_ps[hsl, osl], a2[hsl, :], qT[g][hsl, csl],
                    start=True, stop=True,
                )
        if half is None:
            nc.vector.tensor_copy(u[g][:, QH * qh : QH * (qh + 1)], u_ps[:])
        else:
            hsl = slice(Dh * half, Dh * (half + 1))
            nc.vector.tensor_copy(u[g][hsl, QH * qh : QH * (qh + 1)], u_ps[hsl, :])

    def emit_woF(half):
        """woF chunks 4*half..4*half+3 = blockdiag(Wv^T) @ woTp chunks."""
        f_ps = psum_big.tile([128, 1024], F32, tag="big", name=f"wfps_{it}_{half}")
        for i in range(4):
            c8 = 4 * half + i
            osl = slice(EOUT * i, EOUT * (i + 1))
            nc.tensor.matmul(
                f_ps[0:Dh, osl], wv2[0:Dh, :], woTp[0:Dh, c8, :],
                start=True, stop=True,
            )
            nc.tensor.matmul(
                f_ps[Dh:128, osl], wv2[Dh:128, :], woTp[Dh:128, c8, :],
                start=True, stop=True,
            )
        nc.vector.tensor_copy(
            woF[:, 4 * half : 4 * (half + 1), :].rearrange("p c e -> p (c e)"),
            f_ps[:],
        )

    # ---------------- AllGather staging ----------------
    # pair (pr, qh) staging; the final (1, 1) round is split into two q-512
    # chunks so its serial normalize -> AG -> cch -> oproj chain pipelines.
    in_cc = [
        [dram.tile([2 * Dh, QH], BF16, name=f"incc_{it}_{pr}_{qh}", tag=f"incc{pr}{qh}")
         for qh in range(2)]
        for pr in range(2)
    ]
    ag_outs = [
        [dram.tile([512, QH], BF16, addr_space="Local",
                   name=f"agout_{it}_{pr}_{qh}", tag=f"agout{pr}{qh}")
         for qh in range(2)]
        for pr in range(2)
    ]
    in_cc11 = [
        dram.tile([2 * Dh, 512], BF16, name=f"incc11_{it}_{qc}", tag=f"incc11{qc}")
        for qc in range(2)
    ]
    ag11 = [
        dram.tile([512, 512], BF16, addr_space="Local",
                  name=f"ag11_{it}_{qc}", tag=f"ag11{qc}")
        for qc in range(2)
    ]
    # one tile per 128-row concat chunk: keeps dependency tracking
    # per-chunk so late AllGathers never serialize earlier oproj reads
    cch = [persist.tile([128, S], BF16, tag=f"cch{c8}", name=f"cch{c8}_{it}")
           for c8 in range(8)]

    def emit_ag(pr, qh):
        if collective:
            nc.gpsimd.collective_compute(
                "AllGather",
                mybir.AluOpType.bypass,
                replica_groups=[[0, 1, 2, 3], [4, 5, 6, 7]],
                ins=[in_cc[pr][qh][:, :].opt()],
                outs=[ag_outs[pr][qh].opt()],
            )
        else:
            # sim stand-in: a light dep edge; real AG runs on TOPSP silicon
            nc.sync.dma_start(out=ag_outs[pr][qh][0:128, :], in_=in_cc[pr][qh][:, :])
        for r in range(4):
            eng = (nc.sync, nc.scalar)[r % 2]
            eng.dma_start(
                out=cch[2 * r + pr][:, QH * qh : QH * (qh + 1)],
                in_=ag_outs[pr][qh][128 * r : 128 * (r + 1), :],
            )

    def emit_ag11(qc):
        """Final AllGather, q-512 chunk qc of q-half 1; cch loads spread over
        the SP and DVE DGE queues (both idle in the tail)."""
        if collective:
            nc.gpsimd.collective_compute(
                "AllGather",
                mybir.AluOpType.bypass,
                replica_groups=[[0, 1, 2, 3], [4, 5, 6, 7]],
                ins=[in_cc11[qc][:, :].opt()],
                outs=[ag11[qc].opt()],
            )
        else:
            nc.sync.dma_start(out=ag11[qc][0:128, :], in_=in_cc11[qc][:, :])
        for r in range(4):
            eng = (nc.sync, nc.scalar, nc.gpsimd, nc.sync)[r]
            eng.dma_start(
                out=cch[2 * r + 1][:, QH + 512 * qc : QH + 512 * (qc + 1)],
                in_=ag11[qc][128 * r : 128 * (r + 1), :],
            )

    # ---------------- out projection ----------------
    def emit_oproj(pr, qh):
        """Accumulate parity-`pr` chunks of q-half `qh` into o_acc (pr 0) or
        finish with bias into outT (pr 1)."""
        for h in range(2):
            o_ps = psum_big.tile([128, 1024], F32, tag="big", name=f"ops_{it}_{pr}_{qh}_{h}")
            for t in range(2):
                osl = slice(512 * t, 512 * (t + 1))
                for i, r in enumerate(range(4)):
                    c8 = 2 * r + pr
                    nc.tensor.matmul(
                        o_ps[:, osl],
                        woF[:, c8, 128 * h : 128 * (h + 1)],
                        cch[c8][:, QH * qh + 512 * t : QH * qh + 512 * (t + 1)],
                        start=(i == 0), stop=(i == 3),
                    )
            qsl = slice(QH * qh, QH * (qh + 1))
            if pr == 0:
                nc.vector.tensor_copy(o_acc[h][:, qsl], o_ps[:])
            else:
                o_sb = opool.tile([128, 1024], F32, tag="osb", name=f"osb_{it}_{qh}_{h}")
                nc.vector.scalar_tensor_tensor(
                    o_sb[:], o_ps[:], bo_sb[:, h : h + 1], o_acc[h][:, qsl],
                    mybir.AluOpType.add, mybir.AluOpType.add,
                )
                nc.sync.dma_start(
                    out=outT[128 * h : 128 * (h + 1), qsl], in_=o_sb[:]
                )

    # ---------------- attention stream ----------------
    # All 8 (head, q-half) units flatten into one global stream of 128
    # iterations: sc(i) + exp(i) at iteration i, with the W2 consumer lagging
    # GLOBALLY by W2_LAG iterations so the in-order PE queue never waits on a
    # just-finished exp (the old lag-1 scheme serialized
    # exp -> W2 -> sc -> exp across engines; lag 3 also rides out DVE FIFO latency).
    W2_LAG = 3

    def emit_normalize(j, qh, w2_ps):
        """Normalize chain (executes overlapped with later iterations).
        The row-sum row sits on PSUM partition 64; only ACT can move it to
        partition 0 (DVE lanes are partition-locked, GPSIMD broadcast always
        reads partition 0, DMA cannot read PSUM)."""
        odd = j % 2
        if _DEBUG and j == 0 and qh == 0:
            wraw = persist.tile([Dh + 1, QH], F32, tag="dbg_w2raw_t", name=f"dbgwr_{it}")
            nc.vector.tensor_copy(wraw[:], w2_ps[:, :])
            nc.sync.dma_start(out=io["dbg_w2raw"][:, :], in_=wraw[:])
        rs0 = npool.tile([1, QH], F32, tag="rs0", name=f"rs0_{it}_{j}_{qh}")
        nc.scalar.copy(rs0[:], w2_ps[Dh : Dh + 1, :])
        rsr = npool.tile([1, QH], F32, tag="rsr", name=f"rsr_{it}_{j}_{qh}")
        nc.vector.reciprocal_approx_fast(out=rsr[:], in_=rs0[:])
        rs_b = npool.tile([Dh, QH], F32, tag="rs_b", name=f"rsb_{it}_{j}_{qh}")
        nc.gpsimd.partition_broadcast(rs_b[:], rsr[:])
        w2n = npool.tile([Dh, QH], BF16, tag="w2n", name=f"w2n_{it}_{j}_{qh}")
        nc.vector.tensor_tensor(
            w2n[:], w2_ps[0:Dh, :], rs_b[:], mybir.AluOpType.mult
        )
        if j == 2 and qh == 1:
            # head 2's q-half 1 feeds the SPLIT final-AG staging tiles
            for qc in range(2):
                nc.sync.dma_start(
                    out=in_cc11[qc][0:Dh, :],
                    in_=w2n[:, 512 * qc : 512 * (qc + 1)],
                )
        else:
            nc.sync.dma_start(
                out=in_cc[j // 2][qh][Dh * odd : Dh * (odd + 1), :], in_=w2n[:]
            )
        if _DEBUG and qh == 0 and j in (0, 1):
            nc.sync.dma_start(out=io[f"dbg_w2n{j}0"][:, :], in_=w2n[:])
            if j == 0:
                nc.sync.dma_start(out=io["dbg_rs00"][:, :], in_=rsr[:])
        if odd == 1 and not (j == 3 and qh == 1):
            emit_ag(j // 2, qh)

    def emit_tail(w2_ps):
        """Final unit (head 3, q-half 1): normalize, AllGather, and the last
        out-projection round all split into q-512 chunks so the serial chain
        pipelines; the other three oproj rounds fill the PE meanwhile."""
        o_ps_h = [
            psum_big.tile([128, 1024], F32, tag="big", name=f"opsT_{it}_{h}")
            for h in range(2)
        ]
        for qc in range(2):
            csl = slice(512 * qc, 512 * (qc + 1))
            rs0 = npool.tile([1, 512], F32, tag=f"rs0T{qc}", name=f"rs0T_{it}_{qc}")
            nc.scalar.copy(rs0[:], w2_ps[Dh : Dh + 1, csl])
            rsr = npool.tile([1, 512], F32, tag=f"rsrT{qc}", name=f"rsrT_{it}_{qc}")
            nc.vector.reciprocal_approx_fast(out=rsr[:], in_=rs0[:])
            rs_b = npool.tile([Dh, 512], F32, tag=f"rsbT{qc}", name=f"rsbT_{it}_{qc}")
            nc.gpsimd.partition_broadcast(rs_b[:], rsr[:])
            w2n = npool.tile([Dh, 512], BF16, tag=f"w2nT{qc}", name=f"w2nT_{it}_{qc}")
            nc.vector.tensor_tensor(
                w2n[:], w2_ps[0:Dh, csl], rs_b[:], mybir.AluOpType.mult
            )
            eng = nc.sync if qc == 0 else nc.scalar
            eng.dma_start(out=in_cc11[qc][Dh : 2 * Dh, :], in_=w2n[:])
            emit_ag11(qc)
            if qc == 0:
                emit_oproj(0, 0)
                emit_oproj(0, 1)
                emit_oproj(1, 0)
            for h in range(2):
                for i, r in enumerate(range(4)):
                    nc.tensor.matmul(
                        o_ps_h[h][:, csl],
                        woF[:, 2 * r + 1, 128 * h : 128 * (h + 1)],
                        cch[2 * r + 1][:, QH + 512 * qc : QH + 512 * (qc + 1)],
                        start=(i == 0), stop=(i == 3),
                    )
                o_sb = opool.tile(
                    [128, 512], F32, tag=f"osbT{qc}", name=f"osbT_{it}_{qc}_{h}"
                )
                nc.vector.scalar_tensor_tensor(
                    o_sb[:], o_ps_h[h][:, csl], bo_sb[:, h : h + 1],
                    o_acc[h][:, QH + 512 * qc : QH + 512 * (qc + 1)],
                    mybir.AluOpType.add, mybir.AluOpType.add,
                )
                eng = nc.sync if h == 0 else nc.scalar
                eng.dma_start(
                    out=outT[128 * h : 128 * (h + 1), QH + 512 * qc : QH + 512 * (qc + 1)],
                    in_=o_sb[:],
                )

    # ---------------- schedule ----------------
    emit_u(0, 0, half=0)

    hooks = {
        (0, 0): {1: lambda: emit_u(0, 0, half=1),
                 2: emit_kq_rest0, 6: lambda: emit_vin(1), 10: lambda: emit_u(0, 1)},
        (0, 1): {2: lambda: emit_kq1(0), 6: emit_wot_load, 10: lambda: emit_vin(2)},
        (1, 0): {2: lambda: emit_kq1(1), 5: lambda: emit_u(1, 0),
                 9: lambda: emit_woF(0), 12: lambda: emit_woF(1)},
        (1, 1): {2: lambda: emit_vin(3), 6: lambda: emit_u(1, 1)},
        (2, 0): {},
        (2, 1): {},
        (3, 0): {},
        (3, 1): {},
    }

    units = [(j, qh) for j in range(HPC) for qh in range(2)]
    unit_state = {}        # k -> w2_ps tile
    pend = []              # (emit_w2_closure, post_closure_or_None)

    def drain_one():
        w2c, post = pend.pop(0)
        w2c()
        if post is not None:
            post()

    for k, (j, qh) in enumerate(units):
        g, odd = j // 2, j % 2
        psl = slice(Dh * odd, Dh * (odd + 1))
        w2_ps = psum_acc.tile([Dh + 1, QH], F32, tag="acc", name=f"w2ps_{it}_{j}_{qh}")
        for m in range(NK):
            hk = hooks[(j, qh)].get(m)
            if hk is not None:
                hk()
            sc_ps = psum_big.tile([128, 1024], F32, tag="big", name=f"scps_{it}_{j}_{qh}_{m}")
            for t in range(2):
                nc.tensor.matmul(
                    sc_ps[:, 512 * t : 512 * (t + 1)],
                    kT[g][psl, 128 * m : 128 * (m + 1)],
                    u[g][psl, QH * qh + 512 * t : QH * qh + 512 * (t + 1)],
                    start=True, stop=True,
                )
            p_bf = ppool.tile([128, 1024], BF16, tag="p", name=f"p_{it}_{j}_{qh}_{m}")
            if m in DVE_MS:
                nc.vector.tensor_scalar(
                    p_bf[:].bitcast(I16), sc_ps[:],
                    _EXP_A * 0.125, _EXP_B,
                    mybir.AluOpType.mult, mybir.AluOpType.add,
                )
            else:
                nc.scalar.activation(
                    p_bf[:], sc_ps[:], mybir.ActivationFunctionType.Exp, scale=0.125
                )
            if _DEBUG and j == 0 and qh == 0 and m == 0:
                nc.sync.dma_start(out=io["dbg_p00"][:, :], in_=p_bf[:])

            def w2c(w2_ps=w2_ps, m=m, j=j, p_bf=p_bf):
                for t2 in range(2):
                    nc.tensor.matmul(
                        w2_ps[:, 512 * t2 : 512 * (t2 + 1)],
                        vin_ones[:, m, j, :],
                        p_bf[:, 512 * t2 : 512 * (t2 + 1)],
                        start=(m == 0), stop=(m == NK - 1),
                    )

            post = None
            if m == NK - 1:
                if (j, qh) == (HPC - 1, 1):
                    def post(w2_ps=w2_ps):
                        emit_tail(w2_ps)
                else:
                    def post(j=j, qh=qh, w2_ps=w2_ps):
                        emit_normalize(j, qh, w2_ps)
            pend.append((w2c, post))
            if len(pend) > W2_LAG:
                drain_one()
        # single-buffered W2 psum: drain this unit's W2 tail + normalize NOW
        # so the chain completes before the next unit's first W2 needs the
        # buffer (costs a short PE wait on the last exp, once per unit).
        while pend:
            drain_one()
    if _DEBUG:
        nc.sync.dma_start(out=io["dbg_u0"][:, :], in_=u[0][:, :])
        for c8 in range(8):
            nc.sync.dma_start(
                out=io["dbg_cch"][:, S * c8 : S * (c8 + 1)], in_=cch[c8][:, :]
            )
        nc.sync.dma_start(
            out=io["dbg_woF"][:, :],
            in_=woF[:, :, :].rearrange("p c e -> p (c e)"),
        )
        nc.sync.dma_start(
            out=io["dbg_vones"][:, :],
            in_=vin_ones[:, :, :, :].rearrange("p c j d -> p (c j d)"),
        )


def _build(repeats=1, collective=True):
    key = (repeats, collective)
    if key in _CACHE:
        return _CACHE[key]
    ndev = N_CORES if collective else 1
    nc = bacc.Bacc("TRN2", target_bir_lowering=False, debug=False, num_devices=ndev)
    io = _declare_io(nc)
    with tile.TileContext(nc) as tc:
        for it in range(repeats):
            with contextlib.ExitStack() as es:
                _body(nc, tc, es, io, it, collective=collective)
    nc.compile()
    _CACHE[key] = nc
    return nc


def kernel(k_in, q_in, v_in, Wq, Wk, Wv, Wo, bo, _repeats=1, _results_hook=None):
    import ml_dtypes

    bf16 = ml_dtypes.bfloat16
    k_in = np.asarray(k_in, dtype=np.float32)
    q_in = np.asarray(q_in, dtype=np.float32)
    v_in = np.asarray(v_in, dtype=np.float32)
    Wq = np.ascontiguousarray(np.asarray(Wq, dtype=np.float32))
    Wk = np.ascontiguousarray(np.asarray(Wk, dtype=np.float32))
    Wv = np.ascontiguousarray(np.asarray(Wv, dtype=np.float32))
    Wo = np.asarray(Wo, dtype=np.float32)
    bo = np.asarray(bo, dtype=np.float32)

    nc = _build(_repeats)

    in_maps = []
    for c in range(N_CORES):
        b, q4 = c // 4, c % 4
        sl = slice(256 * q4, 256 * (q4 + 1))
        in_maps.append(
            {
                "qT_s": q_in[b, :, sl].T.astype(bf16),
                "kT_s": k_in[b, :, sl].T.astype(bf16),
                "vin": v_in[b, :, sl].astype(bf16),
                "wq": Wq,
                "wk": Wk,
                "wv": Wv,
                "woT_s": Wo[sl, :].T.astype(bf16),
                "bo_s": np.ascontiguousarray(bo[sl].reshape(2, 128)),
            }
        )

    res = run_bass_kernel_spmd(nc, in_maps, core_ids=list(range(N_CORES)))
    if _results_hook is not None:
        _results_hook(res)

    out = np.empty((B, S, E), dtype=np.float32)
    for c in range(N_CORES):
        b, q4 = c // 4, c % 4
        out[b, :, 256 * q4 : 256 * (q4 + 1)] = res.results[c]["outT"].T
    return out
